# revision 25
# baseline (speedup 1.0000x reference)
"""CubeAttention Trainium2 Bass kernel (8-core SPMD).

Strategy (per sharding hint): data-parallel over the query grid. The 20^3
query grid is split into 8 slabs: 4 blocks along i (x4) times 2 halves
along j (x2). Each core receives a haloed, host-transposed slice of the
padded spatial embeddings plus replicated weights/tables, computes its
[5,10,20,64] output slice fully on-device, and the host reassembles.

Per-core device algorithm (validated in numpy against the reference):
  - KP/VP = slab @ Wk_nb/Wv_nb + bias over the whole haloed slab
    (zero-padding regions give KP=bias, matching reference's k_rel+0+bias).
  - Per 5^3 query block with 9^3 key support:
      logits[q,s] = [Q | C_i | C_j | C_k]^T @ [KP_s | onehot(si,sj,sk)]
    one 91-contraction matmul; C rows carry the separable relpos-key terms
    (shifted G = r@Wk_block matrices) plus the reference's (quirky,
    axis-swapped) edge-validity masks as -1e9 additives.
  - softmax without max-subtraction (max logit ~40 << fp32 overflow),
    normalized via second exp pass with bias = -ln(sum).
  - AV matmul against [VP_s | onehot] yields reweighted values AND the
    three marginal-score tables; the relpos-value contribution is added
    with 15 tiny shifted-RV matmuls accumulating into the same PSUM.
  - out = (AV + fix) @ Wo + bo.
"""

import numpy as np

SCOPE, GN, D, CAP = 2, 20, 64, 32
S3 = 729
NEG = np.float32(-1e9)

_CACHE = {}


def _bass_mod():
    if "nc" in _CACHE:
        return _CACHE["nc"]
    import sys
    for p in ("/opt/trn_rl_repo", "/root/.axon_site/_ro/trn_rl_repo"):
        if p not in sys.path:
            sys.path.append(p)
    import concourse.tile as tile
    from concourse import bacc, mybir
    from concourse.masks import make_identity

    f32 = mybir.dt.float32
    AF = mybir.ActivationFunctionType

    nc = bacc.Bacc("TRN2", target_bir_lowering=False, debug=False)
    P = {}
    P["seT"] = nc.declare_dram_parameter("seT", [64, 3024], f32, isOutput=False)
    P["rT"] = nc.declare_dram_parameter("rT", [32, 5], f32, isOutput=False)
    for nm in ("Wq", "Wknb", "Wvnb", "Wo"):
        P[nm] = nc.declare_dram_parameter(nm, [64, 64], f32, isOutput=False)
    for nm in ("Wk1", "Wk2", "Wk3", "Wv1", "Wv2", "Wv3"):
        P[nm] = nc.declare_dram_parameter(nm, [32, 64], f32, isOutput=False)
    P["bq"] = nc.declare_dram_parameter("bq", [64, 1], f32, isOutput=False)
    P["bk"] = nc.declare_dram_parameter("bk", [64, 1], f32, isOutput=False)
    P["bvb"] = nc.declare_dram_parameter("bvb", [128, 64], f32, isOutput=False)
    P["bob"] = nc.declare_dram_parameter("bob", [125, 64], f32, isOutput=False)
    P["masks"] = nc.declare_dram_parameter("masks", [8, 3, 9, 125], f32, isOutput=False)
    P["ind27"] = nc.declare_dram_parameter("ind27", [27, 729], f32, isOutput=False)
    P["indav"] = nc.declare_dram_parameter("indav", [9, 81, 27], f32, isOutput=False)
    out_p = nc.declare_dram_parameter("out", [5, 10, 20, 64], f32, isOutput=True)
    vps = nc.dram_tensor("vps", [3024, 64], f32)

    with tile.TileContext(nc) as tc:
        with (
            tc.tile_pool(name="const", bufs=1) as const,
            tc.tile_pool(name="big", bufs=1) as big,
            tc.tile_pool(name="work", bufs=2) as work,
            tc.tile_pool(name="psL", bufs=2, space="PSUM") as psL,
            tc.tile_pool(name="psS", bufs=3, space="PSUM") as psS,
            tc.tile_pool(name="psAV", bufs=1, space="PSUM") as psAV,
        ):
            # ---- load constants ----
            sb = {}
            for nm, shp in (
                ("rT", [32, 5]), ("Wq", [64, 64]), ("Wknb", [64, 64]),
                ("Wvnb", [64, 64]), ("Wo", [64, 64]),
                ("Wk1", [32, 64]), ("Wk2", [32, 64]), ("Wk3", [32, 64]),
                ("Wv1", [32, 64]), ("Wv2", [32, 64]), ("Wv3", [32, 64]),
                ("bq", [64, 1]), ("bk", [64, 1]),
                ("bvb", [128, 64]), ("bob", [125, 64]),
            ):
                t = const.tile(shp, f32, tag=nm)
                nc.sync.dma_start(t[:], P[nm][:])
                sb[nm] = t

            seT = big.tile([64, 3024], f32, tag="seT")
            nc.sync.dma_start(seT[:], P["seT"][:])
            seT4 = seT[:].rearrange("p (i j k) -> p i j k", i=9, j=14, k=24)

            ident = const.tile([128, 128], f32, tag="ident")
            make_identity(nc, ident[:])

            # ---- G (key relpos) and RV (value relpos) matrices ----
            Zk = {}
            for ax, wname in (("i", "Wk2"), ("j", "Wk1"), ("k", "Wk3")):
                ps = psS.tile([64, 5], f32, tag="su")
                nc.tensor.matmul(ps[:], sb[wname][:], sb["rT"][:],
                                 start=True, stop=True)
                z = const.tile([64, 13], f32, tag=f"zk{ax}")
                nc.vector.memset(z[:], 0.0)
                nc.vector.tensor_copy(z[:, 4:9], ps[:])
                Zk[ax] = z
            RVSH = {}
            for ax, wname in (("i", "Wv2"), ("j", "Wv1"), ("k", "Wv3")):
                ps = psS.tile([5, 64], f32, tag="su")
                nc.tensor.matmul(ps[:], sb["rT"][:], sb[wname][:],
                                 start=True, stop=True)
                rv5 = const.tile([5, 64], f32, tag=f"rv5{ax}")
                nc.vector.tensor_copy(rv5[:], ps[:])
                for g in range(5):
                    # t[s, :] = RV[s-g] for s-g in [0,5), else 0
                    # (engine partition offsets must be 32-aligned; DMA is not)
                    t = const.tile([9, 64], f32, tag=f"rvsh{ax}{g}")
                    nc.vector.memset(t[:], 0.0)
                    nc.sync.dma_start(t[g:g + 5, :], rv5[:])
                    RVSH[(ax, g)] = t

            # ---- KP projection over slab: KPT [64ch, 3024pos] ----
            KPT = big.tile([64, 3024], f32, tag="KPT")
            for c in range(6):
                ps = psS.tile([64, 504], f32, tag="su")
                nc.tensor.matmul(ps[:], sb["Wknb"][:], seT[:, 504 * c:504 * (c + 1)],
                                 start=True, stop=True)
                nc.scalar.activation(KPT[:, 504 * c:504 * (c + 1)], ps[:],
                                     AF.Identity, bias=sb["bk"][:])
            KPT4 = KPT[:].rearrange("p (i j k) -> p i j k", i=9, j=14, k=24)

            # ---- Q projection: Qall [64ch, 1000] (query order i,j,k) ----
            Qall = big.tile([64, 1000], f32, tag="Qall")
            for i in range(5):
                ps = psS.tile([64, 200], f32, tag="su")
                nc.tensor.matmul(ps[:], sb["Wq"][:], seT4[:, i + 2, 2:12, 2:22],
                                 start=True, stop=True)
                nc.scalar.activation(Qall[:, 200 * i:200 * (i + 1)], ps[:],
                                     AF.Identity, bias=sb["bq"][:])
            Qall4 = Qall[:].rearrange("p (i j k) -> p i j k", i=5, j=10, k=20)

            # ---- VP projection -> DRAM scratch [3024, 64] ----
            for c in range(24):
                n = 128 if c < 23 else 80
                ps = psS.tile([128, 64], f32, tag="su")
                nc.tensor.matmul(ps[:n, :], seT[:, 128 * c:128 * c + n],
                                 sb["Wvnb"][:], start=True, stop=True)
                st = work.tile([128, 64], f32, tag="vpstage")
                nc.vector.tensor_add(st[:n, :], ps[:n, :], sb["bvb"][:n, :])
                nc.sync.dma_start(vps[128 * c:128 * c + n, :], st[:n, :])
            vps4 = vps[:].rearrange("(i j k) c -> i j k c", i=9, j=14, k=24)

            # ---- persistent augmented tiles (ping-pong) ----
            kpa = []
            vpa = []
            for pp in range(2):
                kt = big.tile([91, 729], f32, tag=f"kpa{pp}")
                nc.sync.dma_start(kt[64:91, :], P["ind27"][:])
                kpa.append(kt)
                vt = big.tile([81, 9 * 91], f32, tag=f"vpa{pp}")
                vt3 = vt[:].rearrange("p (s c) -> p s c", s=9)
                nc.sync.dma_start(
                    vt3[:, :, 64:91],
                    P["indav"][:].rearrange("s p c -> p s c"))
                vpa.append(vt)

            # ---- block loop ----
            for blk in range(8):
                bj, bkk = blk // 4, blk % 4
                kp = kpa[blk % 2]
                vp = vpa[blk % 2]
                vp3 = vp[:].rearrange("p (s c) -> p s c", s=9)

                # KP support gather: one DVE copy (engine APs allow 4 dims,
                # and a single producer keeps the QK matmul's sync waits low)
                nc.vector.tensor_copy(
                    kp[0:64, :].rearrange("p (a b c) -> p a b c", a=9, b=9),
                    KPT4[:, :, 5 * bj:5 * bj + 9, 5 * bkk:5 * bkk + 9])
                # VP support gather: one DMA per si-plane; AV matmul si then
                # waits on exactly its own plane's DMA.
                for si in range(9):
                    nc.sync.dma_start(
                        vp3[:, si, 0:64],
                        vps4[si, 5 * bj:5 * bj + 9, 5 * bkk:5 * bkk + 9, :])

                mt3 = work.tile([9, 3 * 125], f32, tag="mask")
                nc.sync.dma_start(
                    mt3[:],
                    P["masks"][blk].rearrange("x s q -> s x q"))

                # Qaug assembly
                qa = work.tile([91, 125], f32, tag="qaug")
                nc.vector.tensor_copy(
                    qa[0:64, :].rearrange("p (a b c) -> p a b c", a=5, b=5),
                    Qall4[:, :, 5 * bj:5 * bj + 5, 5 * bkk:5 * bkk + 5])
                qa3 = qa[:].rearrange("p (a b c) -> p a b c", a=5, b=5)

                csb3 = work.tile([9, 3 * 125], f32, tag="csb")
                for xi, ax in enumerate(("i", "j", "k")):
                    ps = psS.tile([9, 125], f32, tag="su")
                    ps3 = ps[:].rearrange("p (a b c) -> p a b c", a=5, b=5)
                    for g in range(5):
                        lhsT = Zk[ax][:, 4 - g:13 - g]
                        if ax == "i":
                            rhs = qa3[0:64, g, :, :]
                            o = ps3[:, g, :, :]
                        elif ax == "j":
                            rhs = qa3[0:64, :, g, :]
                            o = ps3[:, :, g, :]
                        else:
                            rhs = qa3[0:64, :, :, g]
                            o = ps3[:, :, :, g]
                        nc.tensor.matmul(o, lhsT, rhs, start=True, stop=True)
                    nc.vector.tensor_add(csb3[:, 125 * xi:125 * (xi + 1)],
                                         ps[:], mt3[:, 125 * xi:125 * (xi + 1)])
                # move C rows into Qaug via DMA (partition offsets 73/82 are
                # DMA-only territory; keep SBUF APs partition-major + unsplit)
                for xi in range(3):
                    nc.sync.dma_start(qa[64 + 9 * xi:73 + 9 * xi, :],
                                      csb3[:, 125 * xi:125 * (xi + 1)])

                # QK logits
                psl = psL.tile([125, 729], f32, tag="logits")
                nc.tensor.matmul(psl[:, 0:512], qa[:], kp[:, 0:512],
                                 start=True, stop=True)
                nc.tensor.matmul(psl[:, 512:729], qa[:], kp[:, 512:729],
                                 start=True, stop=True)

                # softmax (no max subtraction; normalize via -ln(Z) bias)
                S = work.tile([125, 729], f32, tag="S")
                Zt = work.tile([125, 1], f32, tag="Zt")
                nc.scalar.activation(S[:], psl[:], AF.Exp)
                nc.vector.reduce_sum(out=Zt[:], in_=S[:], axis=mybir.AxisListType.X)
                lnz = work.tile([125, 1], f32, tag="lnz")
                nc.scalar.activation(lnz[:], Zt[:], AF.Ln)
                nlnz = work.tile([125, 1], f32, tag="nlnz")
                nc.vector.tensor_scalar_mul(nlnz[:], lnz[:], -1.0)
                nc.scalar.activation(S[:], psl[:], AF.Exp, bias=nlnz[:])

                # transpose S per si-plane; AV + marginals accumulate
                psv = psAV.tile([91, 125], f32, tag="av")
                for si in range(9):
                    pst = psS.tile([81, 125], f32, tag="su")
                    nc.tensor.transpose(pst[:], S[:, 81 * si:81 * (si + 1)],
                                        ident[0:125, 0:125])
                    stt = work.tile([81, 125], f32, tag=f"st{si}")
                    if si % 2 == 0:
                        nc.vector.tensor_copy(stt[:], pst[:])
                    else:
                        nc.scalar.copy(stt[:], pst[:])
                    nc.tensor.matmul(psv[:], vp[:, 91 * si:91 * si + 91], stt[:],
                                     start=(si == 0), stop=False)

                # marginals -> SBUF. Engine partition offsets must be
                # 32-aligned (psv[64:91] ok) and DMA cannot read PSUM, so:
                # one aligned DVE copy, then SBUF->SBUF DMAs for the
                # misaligned j/k row groups.
                mall = work.tile([27, 125], f32, tag="mall")
                nc.vector.tensor_copy(mall[:], psv[64:91, :])
                Ms = {"i": mall[0:9, :]}
                for xi, ax in ((1, "j"), (2, "k")):
                    m = work.tile([9, 125], f32, tag=f"m{ax}")
                    nc.sync.dma_start(m[:], mall[9 * xi:9 * xi + 9, :])
                    Ms[ax] = m[:]

                # v_rel fixup into psv rows 0:64
                psv3 = psv[:].rearrange("p (a b c) -> p a b c", a=5, b=5)
                for xi, ax in enumerate(("i", "j", "k")):
                    m3 = Ms[ax].rearrange("p (a b c) -> p a b c", a=5, b=5)
                    for g in range(5):
                        if ax == "i":
                            rhs = m3[:, g, :, :]
                            o = psv3[0:64, g, :, :]
                        elif ax == "j":
                            rhs = m3[:, :, g, :]
                            o = psv3[0:64, :, g, :]
                        else:
                            rhs = m3[:, :, :, g]
                            o = psv3[0:64, :, :, g]
                        nc.tensor.matmul(o, RVSH[(ax, g)][:], rhs,
                                         start=False, stop=(ax == "k" and g == 4))

                avf = work.tile([64, 125], f32, tag="avf")
                nc.vector.tensor_copy(avf[:], psv[0:64, :])

                # out projection + bias + store
                pso = psS.tile([125, 64], f32, tag="su")
                nc.tensor.matmul(pso[:], avf[:], sb["Wo"][:], start=True, stop=True)
                osb = work.tile([125, 64], f32, tag="osb")
                nc.vector.tensor_add(osb[:], pso[:], sb["bob"][:])
                for a in range(5):
                    nc.sync.dma_start(
                        out_p[a, 5 * bj:5 * bj + 5, 5 * bkk:5 * bkk + 5, :],
                        osb[25 * a:25 * (a + 1), :])

    nc.compile()
    _CACHE["nc"] = nc
    return nc


def _host_tables():
    if "tables" in _CACHE:
        return _CACHE["tables"]
    # ind27 [27, 729]
    s = np.arange(S3)
    si, sj, sk = s // 81, (s // 9) % 9, s % 9
    ind27 = np.zeros((27, S3), np.float32)
    for t in range(9):
        ind27[t] = (si == t)
        ind27[9 + t] = (sj == t)
        ind27[18 + t] = (sk == t)
    # indav [9, 81, 27]
    p = np.arange(81)
    pj, pk = p // 9, p % 9
    indav = np.zeros((9, 81, 27), np.float32)
    for plane in range(9):
        indav[plane, :, plane] = 1.0
        for t in range(9):
            indav[plane, :, 9 + t] = (pj == t)
            indav[plane, :, 18 + t] = (pk == t)
    _CACHE["tables"] = (ind27, indav)
    return _CACHE["tables"]


def _masks_for_core(bi, h):
    # [8 blocks, 27, 125] additive masks
    q = np.arange(125)
    a, b, c = q // 25, (q // 5) % 5, q % 5
    sig = np.arange(9)[:, None]          # [9,1]

    def vmask(qx, off):
        # valid iff 2 < qx+off < 22 ; off in [0,5) raw index
        return (qx + off > 2) & (qx + off < 22)

    out = np.zeros((8, 3, 9, 125), np.float32)
    for blk in range(8):
        bj, bkk = blk // 4, blk % 4
        qi = 5 * bi + a
        qj = 10 * h + 5 * bj + b
        qk = 5 * bkk + c
        oi = sig - a[None, :]
        oj = sig - b[None, :]
        ok = sig - c[None, :]
        wi = (oi >= 0) & (oi <= 4)
        wj = (oj >= 0) & (oj <= 4)
        wk = (ok >= 0) & (ok <= 4)
        out[blk, 0] = np.where(wi & vmask(qj[None, :], oi), 0.0, NEG)
        out[blk, 1] = np.where(wj & vmask(qi[None, :], oj), 0.0, NEG)
        out[blk, 2] = np.where(wk & vmask(qk[None, :], ok), 0.0, NEG)
    return out


def kernel(**inputs):
    import sys
    for pth in ("/opt/trn_rl_repo", "/root/.axon_site/_ro/trn_rl_repo"):
        if pth not in sys.path:
            sys.path.append(pth)
    from concourse.bass_utils import run_bass_kernel_spmd

    se = np.asarray(inputs["spatial_embeddings"], np.float32)
    r = np.asarray(inputs["relpos_w"], np.float32)
    Wq = np.asarray(inputs["Wq"], np.float32)
    bq = np.asarray(inputs["bq"], np.float32)
    Wk = np.asarray(inputs["Wk"], np.float32)
    bk = np.asarray(inputs["bk"], np.float32)
    Wv = np.asarray(inputs["Wv"], np.float32)
    bv = np.asarray(inputs["bv"], np.float32)
    Wo = np.asarray(inputs["Wo"], np.float32)
    bo = np.asarray(inputs["bo"], np.float32)

    nc = _bass_mod()
    ind27, indav = _host_tables()

    se_pad = np.pad(se, ((2, 2),) * 3 + ((0, 0),))
    shared = dict(
        rT=np.ascontiguousarray(r.T),
        Wq=Wq, Wknb=np.ascontiguousarray(Wk[96:160]),
        Wvnb=np.ascontiguousarray(Wv[96:160]), Wo=Wo,
        Wk1=np.ascontiguousarray(Wk[0:32]),
        Wk2=np.ascontiguousarray(Wk[32:64]),
        Wk3=np.ascontiguousarray(Wk[64:96]),
        Wv1=np.ascontiguousarray(Wv[0:32]),
        Wv2=np.ascontiguousarray(Wv[32:64]),
        Wv3=np.ascontiguousarray(Wv[64:96]),
        bq=bq.reshape(64, 1), bk=bk.reshape(64, 1),
        bvb=np.ascontiguousarray(np.broadcast_to(bv, (128, 64))),
        bob=np.ascontiguousarray(np.broadcast_to(bo, (125, 64))),
        ind27=ind27, indav=indav,
    )

    in_maps = []
    for core in range(8):
        bi, h = core // 2, core % 2
        slab = se_pad[5 * bi:5 * bi + 9, 10 * h:10 * h + 14, :, :]
        seT = np.ascontiguousarray(slab.transpose(3, 0, 1, 2)).reshape(64, 3024)
        m = dict(shared)
        m["seT"] = seT
        m["masks"] = _masks_for_core(bi, h)
        in_maps.append(m)

    res = run_bass_kernel_spmd(nc, in_maps, core_ids=list(range(8)))
    out = np.empty((20, 20, 20, 64), np.float32)
    for core in range(8):
        bi, h = core // 2, core % 2
        out[5 * bi:5 * bi + 5, 10 * h:10 * h + 10, :, :] = res.results[core]["out"]
    return out


# revision 29
# speedup vs baseline: 3.7996x; 3.7996x over previous
"""CubeAttention Trainium2 Bass kernel (8-core SPMD).

Strategy (per sharding hint): data-parallel over the query grid. The 20^3
query grid is split into 8 slabs: 4 blocks along i (x4) times 2 halves
along j (x2). Each core receives a haloed, host-transposed slice of the
padded spatial embeddings plus replicated weights/tables, computes its
[5,10,20,64] output slice fully on-device, and the host reassembles.

Per-core device algorithm (validated in numpy against the reference):
  - KPT/VPT = Wnb^T @ seT (+bias) over the whole haloed slab, kept in SBUF
    channel-major (zero-padding regions give bias only, matching the
    reference's k_rel + 0 + bias).
  - Per 5^3 query block with 9^3 key support:
      logits[q,s] = [Q | C_i | C_j | C_k]^T @ [KP_s | onehot(si,sj,sk)]
    one 91-contraction matmul; C rows carry the separable relpos-key terms
    (shifted G = r@Wk_block matrices) plus the reference's (quirky,
    axis-swapped) edge-validity masks as -1e9 additives.
  - softmax without max-subtraction (max logit ~40 << fp32 overflow),
    normalized via a second exp pass with bias = -ln(sum).
  - S is PE-transposed per si-plane; the AV matmul against [VP_s | onehot]
    yields reweighted values AND the three marginal-score tables; the
    relpos-value contribution is added with 15 tiny shifted-RV matmuls
    accumulating into the same PSUM. VP support rows are built on-chip by
    PE-transposing VPT (no DRAM roundtrip).
  - out = (AV + fix) @ Wo + bo, one contiguous DMA per block.
"""

import numpy as np

SCOPE, GN, D, CAP = 2, 20, 64, 32
S3 = 729
NEG = np.float32(-1e9)

# column offsets of the packed constant tile [128, 714]
_WP = dict(rT=0, Wq=8, Wknb=72, Wvnb=136, Wo=200,
           Wk1=264, Wk2=328, Wk3=392, Wv1=456, Wv2=520, Wv3=584,
           bq=648, bk=649, bv=650, bob=651)
_WP_COLS = 651 + 64

_CACHE = {}


def _bass_mod(reps=1):
    key = ("nc", reps)
    if key in _CACHE:
        return _CACHE[key]
    import sys
    for p in ("/opt/trn_rl_repo", "/root/.axon_site/_ro/trn_rl_repo"):
        if p not in sys.path:
            sys.path.append(p)
    import concourse.tile as tile
    from concourse import bacc, mybir
    from concourse.masks import make_identity

    f32 = mybir.dt.float32
    AF = mybir.ActivationFunctionType

    nc = bacc.Bacc("TRN2", target_bir_lowering=False, debug=False)
    P = {}
    P["seT"] = nc.declare_dram_parameter("seT", [64, 3024], f32, isOutput=False)
    P["wpack"] = nc.declare_dram_parameter("wpack", [128, _WP_COLS], f32,
                                           isOutput=False)
    P["masks"] = nc.declare_dram_parameter("masks", [8, 3, 9, 125], f32,
                                           isOutput=False)
    P["ind27"] = nc.declare_dram_parameter("ind27", [27, 729], f32,
                                           isOutput=False)
    P["indav"] = nc.declare_dram_parameter("indav", [9, 81, 27], f32,
                                           isOutput=False)
    out_p = nc.declare_dram_parameter("out", [8, 125, 64], f32, isOutput=True)

    with tile.TileContext(nc) as tc:
        with (
            tc.tile_pool(name="const", bufs=1) as const,
            tc.tile_pool(name="big", bufs=1) as big,
            tc.tile_pool(name="work", bufs=2) as work,
            tc.tile_pool(name="psL", bufs=1, space="PSUM") as psL,
            tc.tile_pool(name="psS", bufs=5, space="PSUM") as psS,
            tc.tile_pool(name="psAV", bufs=1, space="PSUM") as psAV,
        ):
            # ---- constants: one packed DMA ----
            wp = const.tile([128, _WP_COLS], f32, tag="wpack")
            nc.sync.dma_start(wp[:], P["wpack"][:])
            sb = {
                "rT": wp[0:32, 0:5],
                "Wq": wp[0:64, 8:72], "Wknb": wp[0:64, 72:136],
                "Wvnb": wp[0:64, 136:200], "Wo": wp[0:64, 200:264],
                "Wk1": wp[0:32, 264:328], "Wk2": wp[0:32, 328:392],
                "Wk3": wp[0:32, 392:456],
                "Wv1": wp[0:32, 456:520], "Wv2": wp[0:32, 520:584],
                "Wv3": wp[0:32, 584:648],
                "bq": wp[0:64, 648:649], "bk": wp[0:64, 649:650],
                "bv": wp[0:64, 650:651], "bob": wp[0:125, 651:715],
            }

            seT = big.tile([64, 3024], f32, tag="seT")
            nc.sync.dma_start(seT[:], P["seT"][:])
            seT4 = seT[:].rearrange("p (i j k) -> p i j k", i=9, j=14, k=24)

            ident = const.tile([128, 128], f32, tag="ident")
            make_identity(nc, ident[:])

            # ---- G (key relpos) and RV (value relpos) matrices ----
            Zk = {}
            for ax, wname in (("i", "Wk2"), ("j", "Wk1"), ("k", "Wk3")):
                ps = psS.tile([64, 5], f32, tag="su")
                nc.tensor.matmul(ps[:], sb[wname], sb["rT"],
                                 start=True, stop=True)
                z = const.tile([64, 13], f32, tag=f"zk{ax}")
                nc.vector.memset(z[:], 0.0)
                nc.vector.tensor_copy(z[:, 4:9], ps[:])
                Zk[ax] = z
            RVSH = {}
            for ax, wname in (("i", "Wv2"), ("j", "Wv1"), ("k", "Wv3")):
                ps = psS.tile([5, 64], f32, tag="su")
                nc.tensor.matmul(ps[:], sb["rT"], sb[wname],
                                 start=True, stop=True)
                rv5 = const.tile([5, 64], f32, tag=f"rv5{ax}")
                nc.vector.tensor_copy(rv5[:], ps[:])
                for g in range(5):
                    # t[s, :] = RV[s-g] for s-g in [0,5), else 0
                    # (engine partition offsets must be 32-aligned; DMA not)
                    t = const.tile([9, 64], f32, tag=f"rvsh{ax}{g}")
                    nc.vector.memset(t[:], 0.0)
                    nc.scalar.dma_start(t[g:g + 5, :], rv5[:])
                    RVSH[(ax, g)] = t

            # ---- KP/VP projections over slab (channel-major, SBUF) ----
            KPT = big.tile([64, 3024], f32, tag="KPT")
            VPT = big.tile([64, 3024], f32, tag="VPT")
            for c in range(6):
                sl = slice(504 * c, 504 * (c + 1))
                ps = psS.tile([64, 504], f32, tag="su")
                nc.tensor.matmul(ps[:], sb["Wknb"], seT[:, sl],
                                 start=True, stop=True)
                nc.scalar.activation(KPT[:, sl], ps[:], AF.Identity,
                                     bias=sb["bk"])
                ps2 = psS.tile([64, 504], f32, tag="su")
                nc.tensor.matmul(ps2[:], sb["Wvnb"], seT[:, sl],
                                 start=True, stop=True)
                nc.scalar.activation(VPT[:, sl], ps2[:], AF.Identity,
                                     bias=sb["bv"])
            KPT4 = KPT[:].rearrange("p (i j k) -> p i j k", i=9, j=14, k=24)
            VPT4 = VPT[:].rearrange("p (i j k) -> p i j k", i=9, j=14, k=24)

            # ---- Q projection: Qall [64ch, 1000] (query order i,j,k) ----
            Qall = big.tile([64, 1000], f32, tag="Qall")
            for i in range(5):
                ps = psS.tile([64, 200], f32, tag="su")
                nc.tensor.matmul(ps[:], sb["Wq"], seT4[:, i + 2, 2:12, 2:22],
                                 start=True, stop=True)
                nc.scalar.activation(Qall[:, 200 * i:200 * (i + 1)], ps[:],
                                     AF.Identity, bias=sb["bq"])
            Qall4 = Qall[:].rearrange("p (i j k) -> p i j k", i=5, j=10, k=20)

            # ---- persistent augmented tiles (ping-pong) ----
            kpa, vpa = [], []
            for pp in range(2):
                kt = big.tile([91, 729], f32, tag=f"kpa{pp}")
                nc.sync.dma_start(kt[64:91, :], P["ind27"][:])
                kpa.append(kt)
                vt = big.tile([81, 9 * 91], f32, tag=f"vpa{pp}")
                vt3 = vt[:].rearrange("p (s c) -> p s c", s=9)
                nc.sync.dma_start(
                    vt3[:, :, 64:91],
                    P["indav"][:].rearrange("s p c -> p s c"))
                vpa.append(vt)

            # ---- block loop, software-pipelined emission ----
            # Stage A(n): support gathers + Qaug/C-row assembly for block n.
            # Stage B(n): QK, softmax, S-transpose, AV+marginals, fixups, out.
            # Emitted A(0), A(1), B(0), A(2), B(1), ... so each engine's
            # program order interleaves blocks and PE never stalls on the
            # C-row DMA chain of the block it is about to compute.

            def stage_A(blk):
                bj, bkk = (blk % 8) // 4, (blk % 8) % 4
                kp = kpa[blk % 2]
                vp = vpa[blk % 2]
                vp3 = vp[:].rearrange("p (s c) -> p s c", s=9)
                jsl = slice(5 * bj, 5 * bj + 9)
                ksl = slice(5 * bkk, 5 * bkk + 9)

                nc.vector.tensor_copy(
                    kp[0:64, :].rearrange("p (a b c) -> p a b c", a=9, b=9),
                    KPT4[:, :, jsl, ksl])
                # VP support rows via PE transpose (matmul RHS needs a single
                # free dim, so stage the gathered support contiguously first)
                vstage = work.tile([64, 729], f32, tag="vstage")
                nc.scalar.copy(
                    vstage[:].rearrange("p (a b c) -> p a b c", a=9, b=9),
                    VPT4[:, :, jsl, ksl])
                for si in range(9):
                    pst = psS.tile([81, 64], f32, tag="su")
                    nc.tensor.transpose(pst[:], vstage[:, 81 * si:81 * (si + 1)],
                                        ident[0:64, 0:64])
                    if si % 2 == 0:
                        nc.vector.tensor_copy(vp3[:, si, 0:64], pst[:])
                    else:
                        nc.scalar.copy(vp3[:, si, 0:64], pst[:])

                mt3 = work.tile([9, 3 * 125], f32, tag="mask")
                nc.scalar.dma_start(
                    mt3[:], P["masks"][blk % 8].rearrange("x s q -> s x q"))

                qa = work.tile([91, 125], f32, tag="qaug")
                nc.vector.tensor_copy(
                    qa[0:64, :].rearrange("p (a b c) -> p a b c", a=5, b=5),
                    Qall4[:, :, 5 * bj:5 * bj + 5, 5 * bkk:5 * bkk + 5])
                qa3 = qa[:].rearrange("p (a b c) -> p a b c", a=5, b=5)

                csb3 = work.tile([9, 3 * 125], f32, tag="csb")
                for xi, ax in enumerate(("i", "j", "k")):
                    ps = psS.tile([9, 125], f32, tag="su")
                    ps3 = ps[:].rearrange("p (a b c) -> p a b c", a=5, b=5)
                    for g in range(5):
                        lhsT = Zk[ax][:, 4 - g:13 - g]
                        if ax == "i":
                            rhs, o = qa3[0:64, g, :, :], ps3[:, g, :, :]
                        elif ax == "j":
                            rhs, o = qa3[0:64, :, g, :], ps3[:, :, g, :]
                        else:
                            rhs, o = qa3[0:64, :, :, g], ps3[:, :, :, g]
                        nc.tensor.matmul(o, lhsT, rhs, start=True, stop=True)
                    nc.vector.tensor_add(csb3[:, 125 * xi:125 * (xi + 1)],
                                         ps[:], mt3[:, 125 * xi:125 * (xi + 1)])
                # move C rows into Qaug (partition offsets 73/82 are DMA-only
                # territory; keep SBUF APs partition-major and unsplit)
                for xi in range(3):
                    nc.sync.dma_start(qa[64 + 9 * xi:73 + 9 * xi, :],
                                      csb3[:, 125 * xi:125 * (xi + 1)])
                return (blk, kp, vp, qa)

            def stage_B(st):
                blk, kp, vp, qa = st
                # QK logits
                psl = psL.tile([125, 729], f32, tag="logits")
                nc.tensor.matmul(psl[:, 0:512], qa[:], kp[:, 0:512],
                                 start=True, stop=True)
                nc.tensor.matmul(psl[:, 512:729], qa[:], kp[:, 512:729],
                                 start=True, stop=True)

                # softmax: exp -> sum -> reciprocal -> per-row scale on ACT
                S = work.tile([125, 729], f32, tag="S")
                Zt = work.tile([125, 1], f32, tag="Zt")
                nc.scalar.activation(S[:], psl[:], AF.Exp)
                nc.vector.reduce_sum(out=Zt[:], in_=S[:],
                                     axis=mybir.AxisListType.X)
                rz = work.tile([125, 1], f32, tag="rz")
                nc.vector.reciprocal(rz[:], Zt[:])
                nc.scalar.activation(S[:], S[:], AF.Identity, scale=rz[:])

                # transpose S per si-plane; AV + marginals accumulate
                psv = psAV.tile([91, 125], f32, tag="av")
                for si in range(9):
                    pst = psS.tile([81, 125], f32, tag="su")
                    nc.tensor.transpose(pst[:], S[:, 81 * si:81 * (si + 1)],
                                        ident[0:125, 0:125])
                    stt = work.tile([81, 125], f32, tag=f"st{si}")
                    if si % 2 == 0:
                        nc.vector.tensor_copy(stt[:], pst[:])
                    else:
                        nc.scalar.copy(stt[:], pst[:])
                    nc.tensor.matmul(psv[:], vp[:, 91 * si:91 * si + 91],
                                     stt[:], start=(si == 0), stop=False)

                # marginals -> SBUF (psv[64:91] is 32-aligned for DVE; the
                # j/k groups are not, so bounce them through DMA)
                mall = work.tile([27, 125], f32, tag="mall")
                nc.vector.tensor_copy(mall[:], psv[64:91, :])
                Ms = {"i": mall[0:9, :]}
                for xi, ax in ((1, "j"), (2, "k")):
                    m = work.tile([9, 125], f32, tag=f"m{ax}")
                    nc.scalar.dma_start(m[:], mall[9 * xi:9 * xi + 9, :])
                    Ms[ax] = m[:]

                # v_rel fixup into psv rows 0:64 (axis i first: its rhs needs
                # no DMA, hiding the j/k marginal-DMA latency)
                psv3 = psv[:].rearrange("p (a b c) -> p a b c", a=5, b=5)
                for xi, ax in enumerate(("i", "j", "k")):
                    m3 = Ms[ax].rearrange("p (a b c) -> p a b c", a=5, b=5)
                    for g in range(5):
                        if ax == "i":
                            rhs, o = m3[:, g, :, :], psv3[0:64, g, :, :]
                        elif ax == "j":
                            rhs, o = m3[:, :, g, :], psv3[0:64, :, g, :]
                        else:
                            rhs, o = m3[:, :, :, g], psv3[0:64, :, :, g]
                        nc.tensor.matmul(o, RVSH[(ax, g)][:], rhs,
                                         start=False,
                                         stop=(ax == "k" and g == 4))

                avf = work.tile([64, 125], f32, tag="avf")
                nc.vector.tensor_copy(avf[:], psv[0:64, :])

                pso = psS.tile([125, 64], f32, tag="su")
                nc.tensor.matmul(pso[:], avf[:], sb["Wo"], start=True, stop=True)
                osb = work.tile([125, 64], f32, tag="osb")
                nc.vector.tensor_add(osb[:], pso[:], sb["bob"])
                nc.gpsimd.dma_start(out_p[blk % 8], osb[:])

            pending = None
            for blk in range(8 * reps):
                st = stage_A(blk)
                if pending is not None:
                    stage_B(pending)
                pending = st
            stage_B(pending)

    nc.compile()
    _CACHE[key] = nc
    _CACHE["nc"] = nc
    return nc


def _host_tables():
    if "tables" in _CACHE:
        return _CACHE["tables"]
    s = np.arange(S3)
    si, sj, sk = s // 81, (s // 9) % 9, s % 9
    ind27 = np.zeros((27, S3), np.float32)
    for t in range(9):
        ind27[t] = (si == t)
        ind27[9 + t] = (sj == t)
        ind27[18 + t] = (sk == t)
    p = np.arange(81)
    pj, pk = p // 9, p % 9
    indav = np.zeros((9, 81, 27), np.float32)
    for plane in range(9):
        indav[plane, :, plane] = 1.0
        for t in range(9):
            indav[plane, :, 9 + t] = (pj == t)
            indav[plane, :, 18 + t] = (pk == t)
    _CACHE["tables"] = (ind27, indav)
    return _CACHE["tables"]


def _masks_for_core(bi, h):
    q = np.arange(125)
    a, b, c = q // 25, (q // 5) % 5, q % 5
    sig = np.arange(9)[:, None]

    def vmask(qx, off):
        return (qx + off > 2) & (qx + off < 22)

    out = np.zeros((8, 3, 9, 125), np.float32)
    for blk in range(8):
        bj, bkk = blk // 4, blk % 4
        qi = 5 * bi + a
        qj = 10 * h + 5 * bj + b
        qk = 5 * bkk + c
        oi = sig - a[None, :]
        oj = sig - b[None, :]
        ok = sig - c[None, :]
        wi = (oi >= 0) & (oi <= 4)
        wj = (oj >= 0) & (oj <= 4)
        wk = (ok >= 0) & (ok <= 4)
        out[blk, 0] = np.where(wi & vmask(qj[None, :], oi), 0.0, NEG)
        out[blk, 1] = np.where(wj & vmask(qi[None, :], oj), 0.0, NEG)
        out[blk, 2] = np.where(wk & vmask(qk[None, :], ok), 0.0, NEG)
    return out


def _pack_weights(inputs):
    Wk, Wv = inputs["Wk"], inputs["Wv"]
    wp = np.zeros((128, _WP_COLS), np.float32)

    def put(name, arr):
        r, c = arr.shape
        wp[0:r, _WP[name]:_WP[name] + c] = arr

    put("rT", inputs["relpos_w"].T)
    put("Wq", inputs["Wq"])
    put("Wknb", Wk[96:160])
    put("Wvnb", Wv[96:160])
    put("Wo", inputs["Wo"])
    put("Wk1", Wk[0:32]); put("Wk2", Wk[32:64]); put("Wk3", Wk[64:96])
    put("Wv1", Wv[0:32]); put("Wv2", Wv[32:64]); put("Wv3", Wv[64:96])
    put("bq", inputs["bq"].reshape(64, 1))
    put("bk", inputs["bk"].reshape(64, 1))
    put("bv", inputs["bv"].reshape(64, 1))
    put("bob", np.broadcast_to(inputs["bo"], (125, 64)))
    return wp


def _make_in_maps(inputs):
    se = np.asarray(inputs["spatial_embeddings"], np.float32)
    inputs = {k: np.asarray(v, np.float32) for k, v in inputs.items()}
    ind27, indav = _host_tables()
    se_pad = np.pad(se, ((2, 2),) * 3 + ((0, 0),))
    shared = dict(wpack=_pack_weights(inputs), ind27=ind27, indav=indav)
    in_maps = []
    for core in range(8):
        bi, h = core // 2, core % 2
        slab = se_pad[5 * bi:5 * bi + 9, 10 * h:10 * h + 14, :, :]
        m = dict(shared)
        m["seT"] = np.ascontiguousarray(slab.transpose(3, 0, 1, 2)).reshape(64, 3024)
        m["masks"] = _masks_for_core(bi, h)
        in_maps.append(m)
    return in_maps


def _assemble(results):
    out = np.empty((20, 20, 20, 64), np.float32)
    for core in range(8):
        bi, h = core // 2, core % 2
        blocks = np.asarray(results[core]["out"]).reshape(8, 5, 5, 5, 64)
        for blk in range(8):
            bj, bkk = blk // 4, blk % 4
            out[5 * bi:5 * bi + 5,
                10 * h + 5 * bj:10 * h + 5 * bj + 5,
                5 * bkk:5 * bkk + 5] = blocks[blk]
    return out


def kernel(**inputs):
    import sys
    for pth in ("/opt/trn_rl_repo", "/root/.axon_site/_ro/trn_rl_repo"):
        if pth not in sys.path:
            sys.path.append(pth)
    from concourse.bass_utils import run_bass_kernel_spmd

    nc = _bass_mod()
    in_maps = _make_in_maps(inputs)
    res = run_bass_kernel_spmd(nc, in_maps, core_ids=list(range(8)))
    return _assemble(res.results)
